# revision 19
# baseline (speedup 1.0000x reference)
"""Trainium2 Bass kernel for nn_BoxLoss (YOLO-style box regression loss).

Contract: kernel(**inputs) takes FULL unsharded inputs (numpy), returns the
FULL scalar loss. Internally: pure data parallel over batch across 8
NeuronCores (4 images per core); each core computes its 12 (scale, image)
row losses entirely on-device and writes a [2,1] partial; the host adds
the 16 partials while unsharding.

Layout: 128 partitions, p = bh*64 + j (image-half, target slot); slots
j in [50,64) are benign padding. Free dim sbl = s*2 + bl (scale,
image-parity), channels innermost.

The indirect-DMA HW consumes ONE index per destination partition
(verified on HW), so the 768 (target, row) cells need 6 gathers; they
are issued stripe-major so each scale's residual chain starts as soon
as its pair of gathers lands. The serial index chain runs wholly on DVE
(cross-engine splits of it cost more in semaphore latency than they
save); Pool computes the independent anchor/validity/area prep and the
cell-index partial, then issues the gathers. Dedup keys are the f32
gather indices themselves, broadcast via PE transpose + DRAM roundtrip,
compared against later targets of the same image half.
"""

import numpy as np

import concourse.bass as bass
import concourse.bacc as bacc
import concourse.mybir as mybir
import concourse.tile as tile
from concourse.tile import add_dep_helper

NCORES = 8
GRIDS = (52, 26, 13)
A = 3           # anchors per scale
T = 50          # targets per image
PB = 4          # images per core
B_TOTAL = 32
HALF = 64       # partition stride of the bh halves
P = 128         # partitions: (bh, j) with 14 pad slots per half
SBL = 6         # free rows: (s, bl)
BIG = float(2 ** 23)   # sentinel key for unmatched targets (> any idx)

F32 = mybir.dt.float32
I32 = mybir.dt.int32

_SCALE_ELEMS = [PB * A * g * g * 85 for g in GRIDS]
_SCALE_BASE = [0, _SCALE_ELEMS[0], _SCALE_ELEMS[0] + _SCALE_ELEMS[1]]
OUTCAT_ELEMS = sum(_SCALE_ELEMS)

# hostpack column layout ([128, _HP_TOT])
_H_TGT = 0        # [0,8)    raw targets (bl, c)
_H_AWH = 8        # [8,44)   anchor w/h (q, sbl, a)
_H_G24 = 44       # [44,68)  g per (sbl, c)
_H_BG = 68        # [68,74)  scale base + b*3*g^2*85  (b = 2bh+bl)
_H_W85 = 74       # [74,80)  85*g
_H_HW85 = 80      # [80,86)  85*g^2
_H_ONESU = 86     # [86,88)  per-half real-row indicators
_H_ONE = 88       # [88,89)  all-ones column
_HP_TOT = 89

# cc inline const [128, _C_TOT]: lat300 | EYE128
_C_LAT = 0        # [0,300)   lat[p, sbl*50+k] = (k > j(p))
_C_EYE = 300      # [300,428) identity-128
_C_TOT = 428


def _host_consts():
    sbl = np.arange(SBL)
    s = sbl // 2
    g = np.array(GRIDS, dtype=np.float64)[s]              # [6]

    g24 = np.broadcast_to(g[:, None], (SBL, 4)).reshape(-1)       # [24]
    w85 = 85.0 * g
    hw85 = 85.0 * g * g
    p = np.arange(P)
    bh = p // HALF
    j = p % HALF
    base = np.array(_SCALE_BASE, dtype=np.float64)[s][None, :]
    b = (2 * bh[:, None] + (sbl % 2)[None, :])
    bg = base + b * (A * 85) * (g ** 2)[None, :]          # [128, 6]
    bg[j >= T, :] = 0.0                                   # pad rows

    row = np.concatenate([
        np.zeros(8), np.zeros(36), g24, np.zeros(6), w85, hw85,
        np.zeros(2), np.ones(1)])
    hp_const = np.broadcast_to(row, (P, _HP_TOT)).copy()
    hp_const[:, _H_BG:_H_BG + 6] = bg
    # benign pad targets: x=0 (invalid), wh=1 (finite rsqrt chain)
    pad = np.tile(np.array([0.0, 0.0, 1.0, 1.0], np.float64), 2)
    hp_const[j >= T, _H_TGT:_H_TGT + 8] = pad
    hp_const[(p < HALF) & (j < T), _H_ONESU] = 1.0
    hp_const[(p >= HALF) & (j < T), _H_ONESU + 1] = 1.0
    return hp_const.astype(np.float32)


def _inline_consts():
    p = np.arange(P)
    j = (p % HALF)[:, None]
    k = np.tile(np.arange(T), SBL)[None, :]
    cc = np.zeros((P, _C_TOT), np.float32)
    cc[:, _C_LAT:_C_LAT + SBL * T] = (k > j)
    cc[:, _C_EYE:_C_EYE + P] = np.eye(P, dtype=np.float32)
    return np.ascontiguousarray(cc)


def build_nc(use_collective: bool = False):
    nc = bacc.Bacc("TRN2", target_bir_lowering=False, debug=False,
                   num_devices=NCORES)

    hp_d = nc.dram_tensor("hostpack", [P, _HP_TOT], F32, kind="ExternalInput")
    outcat_d = nc.dram_tensor("outcat", [OUTCAT_ELEMS], F32, kind="ExternalInput")
    loss_d = nc.dram_tensor("loss", [2, 12], F32, kind="ExternalOutput")
    cc_np = _inline_consts()
    cc_d = nc.inline_tensor(cc_np, name="cc")

    AL = mybir.AluOpType
    AX = mybir.AxisListType.X

    with tile.TileContext(nc) as tc:
        with (
            tc.tile_pool(name="sbuf", bufs=1) as sp,
            tc.tile_pool(name="psum", bufs=1, space="PSUM") as pp,
        ):
            V = nc.vector
            G = nc.gpsimd

            def tt(eng, out, in0, in1, op):
                return eng.tensor_tensor(out=out, in0=in0, in1=in1, op=op)

            def ts(eng, out, in0, s1, op, s2=None, op2=None):
                if op2 is None:
                    return eng.tensor_scalar(out=out, in0=in0, scalar1=s1,
                                             scalar2=None, op0=op)
                return eng.tensor_scalar(out=out, in0=in0, scalar1=s1,
                                         scalar2=s2, op0=op, op1=op2)

            def stt(eng, out, in0, scalar, in1, op0, op1):
                return eng.scalar_tensor_tensor(
                    out=out, in0=in0, scalar=scalar, in1=in1, op0=op0, op1=op1)

            _tn = [0]

            def new(shape, dt=F32):
                _tn[0] += 1
                return sp.tile(shape, dt, name=f"t{_tn[0]}")

            # ---------- input loads ----------
            hp = new([P, _HP_TOT])
            nc.sync.dma_start(out=hp[:], in_=hp_d[:, :])
            cc = new([P, _C_TOT])
            nc.sync.dma_start(out=cc[:], in_=cc_d[:, :])

            def C(c0, w):
                return hp[:, c0:c0 + w]

            tgt = C(_H_TGT, 8)
            awh2 = C(_H_AWH, 36)
            onesU = C(_H_ONESU, 2)
            EYE = cc[:, _C_EYE:_C_EYE + P]
            lat = cc[:, _C_LAT:_C_LAT + SBL * T]


            # ---------- Pool: independent prep ----------
            # padding rows are all-zero; real rows have x in (0.02, 0.98),
            # so x > 0 is an exact validity test for this input family.
            v2 = new([P, 2])
            ts(G, v2[:], tgt.rearrange("p (bl c) -> p bl c", c=4)[:, :, 0:1],
               0.0, AL.is_gt)
            awhh = new([P, 36]); ts(G, awhh[:], awh2, 0.5, AL.mult)
            nawhh = new([P, 36]); ts(G, nawhh[:], awh2, -0.5, AL.mult)
            areaa = new([P, 18])
            tt(G, areaa[:], awh2[:, 0:18], awh2[:, 18:36], AL.mult)

            # ---------- DVE: serial chain to the gather index ----------
            t4 = new([P, 24])
            tt(V, t4[:], tgt[:, None, :].to_broadcast([P, 3, 8]),
               C(_H_G24, 24), AL.mult)
            t4v = t4[:].rearrange("p (sbl c) -> p sbl c", c=4)
            txy = t4v[:, :, 0:2]
            twh = t4v[:, :, 2:4]

            r2 = new([P, 12])
            ts(V, r2[:], txy, float(2 ** 23), AL.add, -float(2 ** 23), AL.add)
            gtm = new([P, 12])
            tt(V, gtm[:], r2[:], txy, AL.is_gt)
            fxy = new([P, 12])
            tt(V, fxy[:], r2[:], gtm[:], AL.subtract)
            zt05 = new([P, 12])
            stt(V, zt05[:], txy, -0.5, fxy[:], AL.add, AL.subtract)

            # Pool: target areas, union partial, and cell-index partial
            areat = new([P, 6])
            tt(G, areat[:], t4v[:, :, 2:3], t4v[:, :, 3:4], AL.mult)
            un1 = new([P, 18])
            tt(G, un1[:], areat[:, :, None].to_broadcast([P, SBL, 3]),
               areaa[:], AL.add)
            fv = fxy[:].rearrange("p (sbl q) -> p sbl q", q=2)
            cx = fv[:, :, 0:1]
            cy = fv[:, :, 1:2]
            iu1 = new([P, 6])
            ts(G, iu1[:], cx, 85.0, AL.mult)
            iu = new([P, 6])
            tt(G, iu[:], iu1[:], C(_H_BG, 6), AL.add)
            iv_ = new([P, 6])
            tt(G, iv_[:], cy, C(_H_W85, 6), AL.mult)
            iw = new([P, 6])
            tt(G, iw[:], iu[:], iv_[:], AL.add)

            # DVE: IoU in (q, sbl, a) layout
            lo = new([P, 12])
            stt(V, lo[:], twh, -0.5, zt05[:], AL.mult, AL.add)
            hi = new([P, 12])
            stt(V, hi[:], twh, 0.5, zt05[:], AL.mult, AL.add)

            def bcQ(t12):
                return (t12[:].rearrange("p (sbl q) -> p q sbl", q=2)
                        [:, :, :, None].to_broadcast([P, 2, SBL, 3]))

            P0 = new([P, 36]); tt(V, P0[:], bcQ(lo), nawhh[:], AL.max)
            P1 = new([P, 36]); tt(V, P1[:], bcQ(hi), awhh[:], AL.min)
            D = new([P, 36]); tt(V, D[:], P1[:], P0[:], AL.subtract)
            M0 = new([P, 36]); ts(V, M0[:], D[:], 0.0, AL.max)
            inter = new([P, 18])
            tt(V, inter[:], M0[:, 0:18], M0[:, 18:36], AL.mult)
            union = new([P, 18])
            tt(V, union[:], un1[:], inter[:], AL.subtract)
            runi = new([P, 18]); V.reciprocal(out=runi[:], in_=union[:])
            iou = new([P, 18]); tt(V, iou[:], inter[:], runi[:], AL.mult)

            iv = iou[:].rearrange("p (sbl a) -> p sbl a", a=3)
            overlap = new([P, 6])
            V.reduce_max(out=overlap[:], in_=iv, axis=AX)
            eqB = new([P, 12])
            tt(V, eqB[:], iv[:, :, 0:2],
               overlap[:, :, None].to_broadcast([P, SBL, 2]), AL.is_equal)
            ev = eqB[:].rearrange("p (sbl e) -> p sbl e", e=2)
            t2 = new([P, 6])
            ts(V, t2[:], ev[:, :, 1:2], 0.0, AL.is_equal, 1.0, AL.add)
            anc = new([P, 6])
            stt(V, anc[:], ev[:, :, 0:1], 0.0, t2[:], AL.is_equal, AL.mult)
            ca = new([P, 6])
            tt(V, ca[:], anc[:], C(_H_HW85, 6), AL.mult)
            idxi = new([P, 6], I32)
            tt(V, idxi[:], ca[:], iw[:], AL.add)
            idxf = new([P, 6])
            tt(V, idxf[:], ca[:], iw[:], AL.add)

            # ---------- Pool: 6 gathers (stripe-major) --------------------
            gpair = [new([P, 8]) for _ in range(3)]
            for q in range(6):
                s_, bl = q // 2, q % 2
                G.indirect_dma_start(
                    out=gpair[s_][:, bl * 4:(bl + 1) * 4], out_offset=None,
                    in_=outcat_d[:].unsqueeze(1),
                    in_offset=bass.IndirectOffsetOnAxis(ap=idxi[:, q:q + 1],
                                                        axis=0),
                )

            # ---------- DVE: dedup keys + PE transpose + DRAM broadcast ---
            om = new([P, 6]); ts(V, om[:], overlap[:], 0.5, AL.is_gt)
            m = new([P, 6])
            tt(V, m[:], om[:].rearrange("p (s bl) -> p s bl", bl=2),
               v2[:, None, :].to_broadcast([P, 3, 2]), AL.mult)
            kk = new([P, 6])
            stt(V, kk[:], idxf[:], -BIG, m[:], AL.add, AL.mult)
            key = new([P, 6]); ts(V, key[:], kk[:], BIG, AL.add)

            keyT_p = pp.tile([SBL, P], F32, name="keyT_p",
                             padded_shape=[128, 512])
            nc.tensor.matmul(out=keyT_p[:], lhsT=key[:], rhs=EYE,
                             start=True, stop=True)
            keyTs = new([SBL, P])
            V.tensor_copy(out=keyTs[:], in_=keyT_p[:])
            kd = nc.dram_tensor("kd", [SBL * P], F32)
            kdw = kd[:].rearrange("(s bh k) -> s bh k", bh=2, k=HALF)
            nc.sync.dma_start(out=kdw[:, :, :], in_=keyTs[:])
            keyB = new([P, SBL * T])
            nc.sync.dma_start(
                out=keyB[0:HALF, :],
                in_=kdw[:, 0, 0:T].unsqueeze(0).to_broadcast([HALF, SBL, T]))
            nc.sync.dma_start(
                out=keyB[HALF:P, :],
                in_=kdw[:, 1, 0:T].unsqueeze(0).to_broadcast([HALF, SBL, T]))

            E = new([P, SBL * T])
            tt(V, E[:], key[:, :, None].to_broadcast([P, SBL, T]),
               keyB[:].rearrange("p (sbl k) -> p sbl k", k=T), AL.is_equal)
            EL = new([P, SBL * T])
            tt(V, EL[:], E[:], lat, AL.mult)
            ov = new([P, 6])
            V.reduce_max(out=ov[:],
                         in_=EL[:].rearrange("p (sbl k) -> p sbl k", k=T),
                         axis=AX)
            winner2 = new([P, 12])  # cols 0:6 winner, cols 6:12 winner*TS
            nov = new([P, 6]); ts(V, nov[:], ov[:], 0.0, AL.is_equal)
            last_dedup = tt(V, winner2[:, 0:6], m[:], nov[:], AL.mult)

            # rsqrt of t_wh (off the critical path)
            rwh2 = new([P, 12])
            V.reciprocal(out=rwh2[:], in_=twh)
            rstw = new([P, 12]); nc.scalar.sqrt(out=rstw[:], in_=rwh2[:])
            rstwv = rstw[:].rearrange("p (sbl q) -> p sbl q", q=2)

            # ---------- per-stripe residuals (pipelined with gathers) -----
            sel = new([P, 24])
            selv = sel[:].rearrange("p (sbl c) -> p sbl c", c=4)
            for s_ in range(3):
                gv = gpair[s_][:].rearrange("p (bl c) -> p bl c", c=4)
                rcpw = new([P, 4])
                ri = V.reciprocal(out=rcpw[:], in_=gv[:, :, 2:4])
                if s_ == 2:
                    # keep the dedup compare ahead of the last stripe in the
                    # DVE stream so it fills the gather window
                    add_dep_helper(ri.ins, last_dedup.ins, True,
                                   "dedup before last stripe")
                rspw = new([P, 4]); nc.scalar.sqrt(out=rspw[:], in_=rcpw[:])
                sx = tt(V, selv[:, 2 * s_:2 * s_ + 2, 0:2], gv[:, :, 0:2],
                        txy[:, 2 * s_:2 * s_ + 2, :], AL.subtract)
                if s_ == 2:
                    add_dep_helper(sx.ins, last_dedup.ins, True,
                                   "dedup before last stripe")
                tt(V, selv[:, 2 * s_:2 * s_ + 2, 2:4],
                   rspw[:].rearrange("p (bl q) -> p bl q", q=2),
                   rstwv[:, 2 * s_:2 * s_ + 2, :], AL.subtract)
            sq = new([P, 24]); tt(V, sq[:], sel[:], sel[:], AL.mult)
            TS2 = new([P, 6])
            V.reduce_sum(out=TS2[:],
                         in_=sq[:].rearrange("p (sbl c) -> p sbl c", c=4),
                         axis=AX)
            tt(V, winner2[:, 6:12], TS2[:], winner2[:, 0:6], AL.mult)

            # ---------- partition reduce + per-row normalize ----------
            M1_p = pp.tile([2, 12], F32, name="M1_p", padded_shape=[128, 512])
            nc.tensor.matmul(out=M1_p[:], lhsT=onesU, rhs=winner2[:],
                             start=True, stop=True)
            M1s = new([2, 12])
            V.tensor_copy(out=M1s[:], in_=M1_p[:])
            nc.sync.dma_start(out=loss_d[:, :], in_=M1s[:])

    nc.compile()
    return nc


_HOST_CONSTS = _host_consts()


def make_in_maps(output0, anchors0, output1, anchors1, output2, anchors2,
                 targets):
    outs = [np.asarray(output0), np.asarray(output1), np.asarray(output2)]
    ancs = [np.asarray(anchors0), np.asarray(anchors1), np.asarray(anchors2)]
    tg = np.asarray(targets)

    # anchor block (q, sbl, a): col = q*18 + (s*2+bl)*3 + a
    awh_row = np.zeros(36, np.float32)
    for q_, col in ((0, 0), (1, 1)):
        for s_ in range(3):
            for bl in range(2):
                for a_ in range(3):
                    awh_row[q_ * 18 + (s_ * 2 + bl) * 3 + a_] = ancs[s_][a_, col]

    in_maps = []
    for c in range(NCORES):
        sl = slice(c * PB, (c + 1) * PB)
        raw = tg[sl, :, 1:5].astype(np.float32)          # [4, 50, 4]
        tg8 = (raw.reshape(2, 2, T, 4)                    # (bh, bl, j, c)
               .transpose(0, 2, 1, 3).reshape(2, T, 8))   # (bh, j) x (bl,c)
        hostpack = _HOST_CONSTS.copy()
        hostpack[0:T, _H_TGT:_H_TGT + 8] = tg8[0]
        hostpack[HALF:HALF + T, _H_TGT:_H_TGT + 8] = tg8[1]
        hostpack[:, _H_AWH:_H_AWH + 36] = awh_row[None, :]
        outcat = np.concatenate([o[sl].ravel() for o in outs]).astype(np.float32)
        in_maps.append({"hostpack": np.ascontiguousarray(hostpack),
                        "outcat": outcat})
    return in_maps


def reduce_loss(m1):
    # m1: [2, 12]; cols 0:6 = n per (bh, sbl), cols 6:12 = sum of winner*TS
    n = np.maximum(np.float32(m1[:, 0:6]), np.float32(1.0)) * np.float32(2.0)
    r = (np.float32(m1[:, 6:12]) / n).astype(np.float32)
    return np.float32(r.sum() / np.float32(B_TOTAL))


_NC_CACHE = {}


def kernel(output0, anchors0, output1, anchors1, output2, anchors2, targets):
    import time
    from concourse.bass_utils import run_bass_kernel_spmd

    if "nc" not in _NC_CACHE:
        _NC_CACHE["nc"] = build_nc(use_collective=False)
    nc = _NC_CACHE["nc"]
    in_maps = make_in_maps(output0, anchors0, output1, anchors1, output2,
                           anchors2, targets)
    res = None
    for attempt in range(3):
        try:
            res = run_bass_kernel_spmd(nc, in_maps, list(range(NCORES)))
            break
        except Exception:
            # transient NRT device errors have been observed; back off + retry
            if attempt == 2:
                raise
            time.sleep(20.0 * (attempt + 1))
    total = np.float32(0.0)
    for c in range(NCORES):
        total += reduce_loss(np.asarray(res.results[c]["loss"]))
    return np.float32(total)


# revision 23
# speedup vs baseline: 1.1614x; 1.1614x over previous
"""Trainium2 Bass kernel for nn_BoxLoss (YOLO-style box regression loss).

Contract: kernel(**inputs) takes FULL unsharded inputs (numpy), returns the
FULL scalar loss. Internally: pure data parallel over batch across 8
NeuronCores (4 images per core); each core computes its 12 (scale, image)
rows and writes the raw per-row [12, 10] reduction; the host finishes the
tiny max(n,1) normalization while unsharding.

Layout: 12 rows r = s*4 + image, 50 targets each; partition
p = r*10 + (j % 10) in [0, 120), column q = j // 10 in [0, 5). (s, b) is
a pure function of the partition, so every constant is a per-partition
hostpack column and the 600 cells need only FIVE indirect gathers (the
indirect-DMA HW consumes ONE index per destination partition - verified).

The serial index chain runs wholly on DVE; Pool computes the independent
anchor/validity/area prep and the cell-index partial, then issues the
gathers. Dedup keys are the f32 gather indices (exact through the PE
transpose), broadcast back via one DRAM roundtrip with a row-grouped
factored-partition access pattern, and compared against later targets of
the same row.
"""

import numpy as np

import concourse.bass as bass
import concourse.bacc as bacc
import concourse.mybir as mybir
import concourse.tile as tile
from concourse.tile import add_dep_helper

NCORES = 8
GRIDS = (52, 26, 13)
A = 3           # anchors per scale
T = 50          # targets per image
PB = 4          # images per core
B_TOTAL = 32
R = 12          # rows per core: (scale, image)
NQ = 5          # target groups (columns) per row
P = 120         # partitions: r*10 + j%10
BIG = float(2 ** 23)   # sentinel key for unmatched targets (> any idx)

F32 = mybir.dt.float32
I32 = mybir.dt.int32

_SCALE_ELEMS = [PB * A * g * g * 85 for g in GRIDS]
_SCALE_BASE = [0, _SCALE_ELEMS[0], _SCALE_ELEMS[0] + _SCALE_ELEMS[1]]
OUTCAT_ELEMS = sum(_SCALE_ELEMS)

# hostpack column layout ([120, _HP_TOT])
_H_TGT = 0        # [0,20)   raw targets (q, c)
_H_AWH = 20       # [20,50)  anchor w/h (d, q, a)
_H_G20 = 50       # [50,70)  g per (q, c)
_H_BG = 70        # [70,75)  scale base + b*3*g^2*85 (same for all q)
_H_W85 = 75       # [75,80)  85*g
_H_HW85 = 80      # [80,85)  85*g^2
_H_ONESR = 85     # [85,97)  row indicator (p//10 == r)
_HP_TOT = 97

# cc inline const [120, _C_TOT]: latB 250 | EYE 120
_C_LAT = 0        # [0,250)  latB[p,(q,k)] = (k > q*10 + p%10)
_C_EYE = 250      # [250,370) identity-120
_C_TOT = 370


def _host_consts():
    p = np.arange(P)
    r = p // 10
    s = r // PB
    g = np.array(GRIDS, dtype=np.float64)[s]              # [120]
    b = r % PB
    base = np.array(_SCALE_BASE, dtype=np.float64)[s]
    bg = base + b * (A * 85) * g * g                      # [120]

    hp = np.zeros((P, _HP_TOT), np.float64)
    hp[:, _H_G20:_H_G20 + 20] = g[:, None]
    hp[:, _H_BG:_H_BG + NQ] = bg[:, None]
    hp[:, _H_W85:_H_W85 + NQ] = (85.0 * g)[:, None]
    hp[:, _H_HW85:_H_HW85 + NQ] = (85.0 * g * g)[:, None]
    hp[np.arange(P), _H_ONESR + r] = 1.0
    return hp.astype(np.float32)


def _inline_consts():
    p = np.arange(P)
    jlow = (p % 10)[:, None]
    q = np.repeat(np.arange(NQ), T)[None, :]      # (q, c) flattened
    c = np.tile(np.arange(T), NQ)[None, :]
    # keyB arrives rotated: partition p reads kd window starting at 5*(p%10),
    # so column c holds target k = (5*(p%10) + c) % 50
    k = (5 * jlow + c) % T
    cc = np.zeros((P, _C_TOT), np.float32)
    cc[:, _C_LAT:_C_LAT + NQ * T] = (k > q * 10 + jlow)
    cc[:, _C_EYE:_C_EYE + P] = np.eye(P, dtype=np.float32)
    return np.ascontiguousarray(cc)


def build_nc(use_collective: bool = False):
    nc = bacc.Bacc("TRN2", target_bir_lowering=False, debug=False,
                   num_devices=NCORES)

    hp_d = nc.dram_tensor("hostpack", [P, _HP_TOT], F32, kind="ExternalInput")
    outcat_d = nc.dram_tensor("outcat", [OUTCAT_ELEMS], F32, kind="ExternalInput")
    loss_d = nc.dram_tensor("loss", [R, 10], F32, kind="ExternalOutput")
    cc_np = _inline_consts()
    cc_d = nc.inline_tensor(cc_np, name="cc")

    AL = mybir.AluOpType
    AX = mybir.AxisListType.X

    with tile.TileContext(nc) as tc:
        with (
            tc.tile_pool(name="sbuf", bufs=1) as sp,
            tc.tile_pool(name="psum", bufs=1, space="PSUM") as pp,
        ):
            V = nc.vector
            G = nc.gpsimd

            def tt(eng, out, in0, in1, op):
                return eng.tensor_tensor(out=out, in0=in0, in1=in1, op=op)

            def ts(eng, out, in0, s1, op, s2=None, op2=None):
                if op2 is None:
                    return eng.tensor_scalar(out=out, in0=in0, scalar1=s1,
                                             scalar2=None, op0=op)
                return eng.tensor_scalar(out=out, in0=in0, scalar1=s1,
                                         scalar2=s2, op0=op, op1=op2)

            def stt(eng, out, in0, scalar, in1, op0, op1):
                return eng.scalar_tensor_tensor(
                    out=out, in0=in0, scalar=scalar, in1=in1, op0=op0, op1=op1)

            _tn = [0]

            def new(shape, dt=F32):
                _tn[0] += 1
                return sp.tile(shape, dt, name=f"t{_tn[0]}")

            # ---------- input loads ----------
            hp = new([P, _HP_TOT])
            nc.sync.dma_start(out=hp[:], in_=hp_d[:, :])
            cc = new([P, _C_TOT])
            nc.scalar.dma_start(out=cc[:], in_=cc_d[:, :])

            def C(c0, w):
                return hp[:, c0:c0 + w]

            tgt = C(_H_TGT, 20)
            tgtv = tgt.rearrange("p (q c) -> p q c", c=4)
            awh2 = C(_H_AWH, 30)
            onesR = C(_H_ONESR, R)
            EYE = cc[:, _C_EYE:_C_EYE + P]
            latB = cc[:, _C_LAT:_C_LAT + NQ * T]

            # ---------- Pool: independent prep ----------
            # padding rows are all-zero; real rows have x in (0.02, 0.98),
            # so x > 0 is an exact validity test for this input family.
            xv = new([P, NQ])
            ts(G, xv[:], tgtv[:, :, 0:1], 0.0, AL.is_gt)
            awhh = new([P, 30]); ts(G, awhh[:], awh2, 0.5, AL.mult)
            nawhh = new([P, 30]); ts(G, nawhh[:], awh2, -0.5, AL.mult)
            areaa = new([P, 15])
            tt(G, areaa[:], awh2[:, 0:15], awh2[:, 15:30], AL.mult)

            # ---------- DVE: serial chain to the gather index ----------
            t4 = new([P, 20])
            tt(V, t4[:], tgt, C(_H_G20, 20), AL.mult)
            t4v = t4[:].rearrange("p (q c) -> p q c", c=4)
            txy = t4v[:, :, 0:2]
            twh = t4v[:, :, 2:4]

            r2 = new([P, 10])
            ts(V, r2[:], txy, float(2 ** 23), AL.add, -float(2 ** 23), AL.add)
            gtm = new([P, 10])
            tt(V, gtm[:], r2[:], txy, AL.is_gt)
            fxy = new([P, 10])
            tt(V, fxy[:], r2[:], gtm[:], AL.subtract)
            zt05 = new([P, 10])
            stt(V, zt05[:], txy, -0.5, fxy[:], AL.add, AL.subtract)

            # Pool: target areas, union partial, and cell-index partial
            areat = new([P, NQ])
            tt(G, areat[:], t4v[:, :, 2:3], t4v[:, :, 3:4], AL.mult)
            un1 = new([P, 15])
            tt(G, un1[:], areat[:, :, None].to_broadcast([P, NQ, A]),
               areaa[:], AL.add)
            fv = fxy[:].rearrange("p (q d) -> p q d", d=2)
            cx = fv[:, :, 0:1]
            cy = fv[:, :, 1:2]
            iu1 = new([P, NQ])
            ts(G, iu1[:], cx, 85.0, AL.mult)
            iu = new([P, NQ])
            tt(G, iu[:], iu1[:], C(_H_BG, NQ), AL.add)
            iv_ = new([P, NQ])
            tt(G, iv_[:], cy, C(_H_W85, NQ), AL.mult)
            iw = new([P, NQ])
            tt(G, iw[:], iu[:], iv_[:], AL.add)

            # DVE: IoU in (d, q, a) layout
            lo = new([P, 10])
            stt(V, lo[:], twh, -0.5, zt05[:], AL.mult, AL.add)
            hi = new([P, 10])
            stt(V, hi[:], twh, 0.5, zt05[:], AL.mult, AL.add)

            def bcQ(t10):
                return (t10[:].rearrange("p (q d) -> p d q", d=2)
                        [:, :, :, None].to_broadcast([P, 2, NQ, A]))

            P0 = new([P, 30]); tt(V, P0[:], bcQ(lo), nawhh[:], AL.max)
            P1 = new([P, 30]); tt(V, P1[:], bcQ(hi), awhh[:], AL.min)
            D = new([P, 30]); tt(V, D[:], P1[:], P0[:], AL.subtract)
            M0 = new([P, 30]); ts(V, M0[:], D[:], 0.0, AL.max)
            inter = new([P, 15])
            tt(V, inter[:], M0[:, 0:15], M0[:, 15:30], AL.mult)
            union = new([P, 15])
            tt(V, union[:], un1[:], inter[:], AL.subtract)
            runi = new([P, 15]); V.reciprocal(out=runi[:], in_=union[:])
            iou = new([P, 15]); tt(V, iou[:], inter[:], runi[:], AL.mult)

            iv = iou[:].rearrange("p (q a) -> p q a", a=A)
            overlap = new([P, NQ])
            V.reduce_max(out=overlap[:], in_=iv, axis=AX)
            eqB = new([P, 10])
            tt(V, eqB[:], iv[:, :, 0:2],
               overlap[:, :, None].to_broadcast([P, NQ, 2]), AL.is_equal)
            ev = eqB[:].rearrange("p (q e) -> p q e", e=2)
            t2 = new([P, NQ])
            ts(V, t2[:], ev[:, :, 1:2], 0.0, AL.is_equal, 1.0, AL.add)
            anc = new([P, NQ])
            stt(V, anc[:], ev[:, :, 0:1], 0.0, t2[:], AL.is_equal, AL.mult)
            ca = new([P, NQ])
            tt(V, ca[:], anc[:], C(_H_HW85, NQ), AL.mult)
            idxi = new([P, NQ], I32)
            tt(V, idxi[:], ca[:], iw[:], AL.add)
            idxf = new([P, NQ])
            tt(V, idxf[:], ca[:], iw[:], AL.add)

            # ---------- Pool: 5 gathers ----------
            g20 = new([P, 20])
            for q in range(NQ):
                G.indirect_dma_start(
                    out=g20[:, q * 4:(q + 1) * 4], out_offset=None,
                    in_=outcat_d[:].unsqueeze(1),
                    in_offset=bass.IndirectOffsetOnAxis(ap=idxi[:, q:q + 1],
                                                        axis=0),
                )

            # ---------- DVE: dedup keys + PE transpose + DRAM broadcast ---
            om = new([P, NQ]); ts(V, om[:], overlap[:], 0.5, AL.is_gt)
            m = new([P, NQ])
            tt(V, m[:], om[:], xv[:], AL.mult)
            kk = new([P, NQ])
            stt(V, kk[:], idxf[:], -BIG, m[:], AL.add, AL.mult)
            key = new([P, NQ]); ts(V, key[:], kk[:], BIG, AL.add)

            keyT_p = pp.tile([NQ, P], F32, name="keyT_p",
                             padded_shape=[128, 512])
            nc.tensor.matmul(out=keyT_p[:], lhsT=key[:], rhs=EYE,
                             start=True, stop=True)
            keyTs = new([NQ, P])
            V.tensor_copy(out=keyTs[:], in_=keyT_p[:])
            # kd stores each row's 50 keys twice: kd[r*100 + {0,50} + k],
            # so partition p = r*10+pi can read its row's keys from the
            # affine overlapping window kd[5*p : 5*p + 50] (rotated by 5*pi;
            # the rotation is baked into the latB mask)
            kd = nc.dram_tensor("kd", [R * 2 * T], F32)
            kdv = kd[:].rearrange("(r c) -> r c", c=2 * T)
            keyTs3 = keyTs[:].rearrange("kq (r kr) -> kq r kr", kr=10)
            nc.sync.dma_start(
                out=kdv[:, 0:T].rearrange("r (kq kr) -> kq r kr", kr=10),
                in_=keyTs3)
            nc.scalar.dma_start(
                out=kdv[:, T:2 * T].rearrange("r (kq kr) -> kq r kr", kr=10),
                in_=keyTs3)
            keyB = new([P, T])
            nc.sync.dma_start(
                out=keyB[:], in_=bass.AP(kd, 0, [[5, P], [1, T]]))

            E = new([P, NQ * T])
            tt(V, E[:], key[:, :, None].to_broadcast([P, NQ, T]),
               keyB[:, None, :].to_broadcast([P, NQ, T]), AL.is_equal)
            EL = new([P, NQ * T])
            tt(V, EL[:], E[:], latB, AL.mult)
            ov = new([P, NQ])
            V.reduce_max(out=ov[:],
                         in_=EL[:].rearrange("p (q k) -> p q k", k=T),
                         axis=AX)
            winner2 = new([P, 10])  # cols 0:5 winner, cols 5:10 winner*TS
            nov = new([P, NQ]); ts(V, nov[:], ov[:], 0.0, AL.is_equal)
            last_dedup = tt(V, winner2[:, 0:NQ], m[:], nov[:], AL.mult)

            # rsqrt of t_wh (off the critical path)
            rwh2 = new([P, 10])
            V.reciprocal(out=rwh2[:], in_=twh)
            rstw = new([P, 10]); nc.scalar.sqrt(out=rstw[:], in_=rwh2[:])
            rstwv = rstw[:].rearrange("p (q d) -> p q d", d=2)

            # ---------- residuals in two chunks (q 0:3 | q 3:5) ----------
            sel = new([P, 20])
            selv = sel[:].rearrange("p (q c) -> p q c", c=4)
            gv = g20[:].rearrange("p (q c) -> p q c", c=4)
            for lo_q, hi_q in ((0, 3), (3, NQ)):
                w = hi_q - lo_q
                rcpw = new([P, 2 * w])
                ri = V.reciprocal(out=rcpw[:], in_=gv[:, lo_q:hi_q, 2:4])
                if lo_q == 3:
                    # keep the dedup compare ahead of the last chunk in the
                    # DVE stream so it fills the gather window
                    add_dep_helper(ri.ins, last_dedup.ins, True,
                                   "dedup before last chunk")
                rspw = new([P, 2 * w]); nc.scalar.sqrt(out=rspw[:], in_=rcpw[:])
                sx = tt(V, selv[:, lo_q:hi_q, 0:2], gv[:, lo_q:hi_q, 0:2],
                        txy[:, lo_q:hi_q, :], AL.subtract)
                if lo_q == 3:
                    add_dep_helper(sx.ins, last_dedup.ins, True,
                                   "dedup before last chunk")
                tt(V, selv[:, lo_q:hi_q, 2:4],
                   rspw[:].rearrange("p (q d) -> p q d", d=2),
                   rstwv[:, lo_q:hi_q, :], AL.subtract)
            sq = new([P, 20]); tt(V, sq[:], sel[:], sel[:], AL.mult)
            TS2 = new([P, NQ])
            V.reduce_sum(out=TS2[:],
                         in_=sq[:].rearrange("p (q c) -> p q c", c=4),
                         axis=AX)
            tt(V, winner2[:, NQ:10], TS2[:], winner2[:, 0:NQ], AL.mult)

            # ---------- per-row partition reduce ----------
            M1_p = pp.tile([R, 10], F32, name="M1_p", padded_shape=[128, 512])
            nc.tensor.matmul(out=M1_p[:], lhsT=onesR, rhs=winner2[:],
                             start=True, stop=True)
            M1s = new([R, 10])
            V.tensor_copy(out=M1s[:], in_=M1_p[:])
            nc.sync.dma_start(out=loss_d[:, :], in_=M1s[:])

    nc.compile()
    return nc


_HOST_CONSTS = _host_consts()


def make_in_maps(output0, anchors0, output1, anchors1, output2, anchors2,
                 targets):
    outs = [np.asarray(output0), np.asarray(output1), np.asarray(output2)]
    ancs = [np.asarray(anchors0), np.asarray(anchors1), np.asarray(anchors2)]
    tg = np.asarray(targets)

    # anchors per partition: awh[p, (d, q, a)] = ancs[s(p)][a, d]
    p = np.arange(P)
    s_of_p = (p // 10) // PB
    awh = np.zeros((P, 30), np.float32)
    for d in range(2):
        for q_ in range(NQ):
            for a_ in range(A):
                col = d * 15 + q_ * 3 + a_
                awh[:, col] = np.array(
                    [ancs[s][a_, d] for s in s_of_p], np.float32)

    in_maps = []
    for c in range(NCORES):
        sl = slice(c * PB, (c + 1) * PB)
        raw = tg[sl, :, 1:5].astype(np.float32)          # [4, 50, 4]
        hostpack = _HOST_CONSTS.copy()
        # row r = s*4 + bi; partition r*10 + j%10; col (j//10)*4 + c
        rows = raw.reshape(1, PB, T, 4)
        for s_ in range(3):
            for bi in range(PB):
                r = s_ * PB + bi
                t50 = rows[0, bi]                         # [50, 4]
                t5 = t50.reshape(NQ, 10, 4).transpose(1, 0, 2).reshape(10, 20)
                hostpack[r * 10:(r + 1) * 10, _H_TGT:_H_TGT + 20] = t5
        hostpack[:, _H_AWH:_H_AWH + 30] = awh
        outcat = np.concatenate([o[sl].ravel() for o in outs]).astype(np.float32)
        in_maps.append({"hostpack": np.ascontiguousarray(hostpack),
                        "outcat": outcat})
    return in_maps


def reduce_loss(m1):
    # m1: [12, 10]; cols 0:5 = n per (r, q), cols 5:10 = sum winner*TS
    m1 = np.float32(m1)
    n = np.maximum(m1[:, 0:NQ].sum(axis=1, dtype=np.float32), np.float32(1.0))
    ts = m1[:, NQ:10].sum(axis=1, dtype=np.float32)
    return np.float32((ts / (n * np.float32(2.0))).sum() / np.float32(B_TOTAL))


_NC_CACHE = {}


def kernel(output0, anchors0, output1, anchors1, output2, anchors2, targets):
    import time
    from concourse.bass_utils import run_bass_kernel_spmd

    if "nc" not in _NC_CACHE:
        _NC_CACHE["nc"] = build_nc(use_collective=False)
    nc = _NC_CACHE["nc"]
    in_maps = make_in_maps(output0, anchors0, output1, anchors1, output2,
                           anchors2, targets)
    res = None
    for attempt in range(3):
        try:
            res = run_bass_kernel_spmd(nc, in_maps, list(range(NCORES)))
            break
        except Exception:
            # transient NRT device errors have been observed; back off + retry
            if attempt == 2:
                raise
            time.sleep(20.0 * (attempt + 1))
    total = np.float32(0.0)
    for c in range(NCORES):
        total += reduce_loss(np.asarray(res.results[c]["loss"]))
    return np.float32(total)
